# revision 47
# baseline (speedup 1.0000x reference)
"""Trainium2 Bass kernel for segment-reduce classifier.

Reference computation:
    local = relu(x @ Wloc.T)            # [L, 128]
    feats = local.reshape(-1, 30, 128).mean(1)   # [L/30, 128]
    out   = feats @ W.T                 # [L/30, 10]

Strategy v4 (8 NeuronCores, data-parallel on rows):
  - Per core xt [128, 75000] fp16: partitions 0-63 = rows[:75000].T ("A"),
    64-127 = rows[75000:].T ("B").  Within each outer tile (7680 rows =
    256 segments per stream) the host pre-permutes rows to tile-scope
    j-major (r = j*gt + g), so every on-chip access pattern is contiguous:
    mm1 streams flat 512-col chunks, relu reads flat PSUM, and mm2's per-j
    operand is two contiguous gt-element runs.
  - mm1: each [128, 1024] PSUM tile holds one A-chunk (bank 0, PE rows
    0-63) and one B-chunk (bank 1, rows 64-127); the two matmuls are
    adjacent in the queue and run concurrently on disjoint row quadrants.
    ppp bufs=3 keeps the PE filling tile k+1 while relus drain k/k-1.
  - relu PSUM fp32 -> SBUF fp16, one flat [p, 1024] instruction per pp
    tile, engines alternating Scalar(8) : Vector(7) (~1 elem/lane/cycle
    each on fp32 PSUM reads; this pass is the ~80us/core roofline).
  - mm2 (mean-pool + classifier fused): per outer tile, 30 accumulating
    512-col matmuls (rhs = rl[:, h, j*gt:(j+1)*gt]) into a per-tile PSUM
    accumulator [10, 2*gt] at column-group 32*(t%4); pooling is free PSUM
    accumulation, 1/30 folded into W.  js are deferred and dripped two
    per slot from the two oldest pending tiles, so adjacent mm2s use
    different column groups and pair up on the PE.
  - Per tile: one [10, 2*gt] copy PSUM->SBUF (alternating engines) + DMA.
"""

import numpy as np

import concourse.bacc as bacc
import concourse.bass as bass
import concourse.tile as tile
from concourse import mybir
from concourse.bass_utils import run_bass_kernel_spmd

# Problem constants (hardcoded per harness contract)
L, D_IN, D_ENC, C, J = 1200000, 64, 128, 10, 30
N_CORES = 8
R = L // N_CORES          # rows per core = 150000
HALF = R // 2             # 75000 rows per quadrant stream
SEG_HALF = HALF // J      # 2500 segments per stream
TFS = [7680] * 9 + [5880]  # rows per outer tile per stream
CH = 512                  # rows per PSUM bank chunk

_CACHE = {}


def _build_kernel():
    nc = bacc.Bacc("TRN2", target_bir_lowering=False, debug=False,
                   num_devices=N_CORES)
    f32, f16 = mybir.dt.float32, mybir.dt.float16

    xt_d = nc.dram_tensor("xt", [128, HALF], f16, kind="ExternalInput")
    w1_d = nc.dram_tensor("w1", [128, D_ENC], f16, kind="ExternalInput")
    w2_d = nc.dram_tensor("w2", [128, C], f16, kind="ExternalInput")
    # tile t at cols [512t, 512t + 2*gt); half h at h*gt + seg
    out_d = nc.dram_tensor("out", [C, 512 * len(TFS)], f32,
                           kind="ExternalOutput")

    with tile.TileContext(nc) as tc:
        with (
            tc.tile_pool(name="consts", bufs=1) as consts,
            tc.tile_pool(name="xin", bufs=4) as xin,
            tc.tile_pool(name="xin0", bufs=4) as xin0,
            tc.tile_pool(name="rlcp", bufs=10) as rlcp,
            tc.tile_pool(name="stgp", bufs=2) as stgp,
            tc.tile_pool(name="ppp", bufs=3, space="PSUM") as ppp,
            tc.tile_pool(name="accp", bufs=2, space="PSUM") as accp,
        ):
            w1 = consts.tile([128, D_ENC], f16)
            nc.sync.dma_start(w1[:], w1_d[:])
            w2 = consts.tile([128, C], f16)
            nc.sync.dma_start(w2[:], w2_d[:])

            pp_i = 0       # global relu counter (engine pattern, 8A:7D)
            pends = []     # lists of deferred mm2 emitters, one per tile
            rr = [0]       # round-robin cursor over pends

            def make_mm2(rl_t, tf, t, rlcs=None):
                gt = tf // J
                hgt = gt // 2
                if rlcs is None:
                    rlh = rl_t.rearrange("p (h q) -> p h q", h=2)
                box = {}

                def emit(j):
                    if j == 0:
                        box["acc"] = accp.tile([128, 512], f32, tag="acc",
                                               name="acc")
                    acc = box["acc"]
                    # 4 concurrent column-group quarters per j; out col
                    # q*hgt == h*gt + (q%2)*hgt, same layout as one wide mm
                    for q in range(4):
                        so = j * gt + (q % 2) * hgt
                        if rlcs is None:
                            nc.tensor.matmul(
                                acc[32 * q:32 * q + C,
                                    q * hgt:(q + 1) * hgt],
                                w2[:], rlh[:, q // 2, so:so + hgt],
                                start=(j == 0), stop=(j == J - 1),
                                tile_position=(0, 32 * q),
                                skip_group_check=True)
                        else:
                            # per-chunk rl tiles; reads may straddle banks
                            a, outo, left = so, q * hgt, hgt
                            while left > 0:
                                ck, off = a // CH, a % CH
                                take = min(left, CH - off)
                                nc.tensor.matmul(
                                    acc[32 * q:32 * q + C,
                                        outo:outo + take],
                                    w2[:],
                                    rlcs[ck][:, (q // 2) * CH + off:
                                             (q // 2) * CH + off + take],
                                    start=(j == 0), stop=(j == J - 1),
                                    tile_position=(0, 32 * q),
                                    skip_group_check=True)
                                a += take
                                outo += take
                                left -= take
                    return

                def post(half):
                    # deferred drain, dripped a few slots after j=29 so
                    # the copying engine never blocks on the PE
                    if half == 1:
                        return
                    acc = box["acc"]
                    stg = stgp.tile([128, 512], f32, tag="stg",
                                    name="stg")
                    if t % 2 == 1:
                        nc.scalar.copy(stg[:, 0:2 * gt], acc[:, 0:2 * gt])
                    else:
                        nc.vector.tensor_scalar_add(
                            stg[:, 0:2 * gt], acc[:, 0:2 * gt], 0.0)
                    for q in range(4):
                        nc.sync.dma_start(
                            out_d[:, 512 * t + q * hgt:
                                  512 * t + (q + 1) * hgt],
                            stg[32 * q:32 * q + C,
                                q * hgt:(q + 1) * hgt])
                return emit, post

            def drip(n):
                for _ in range(n):
                    live = [d for d in pends if d]
                    if not live:
                        return
                    d = live[rr[0] % len(live)]
                    rr[0] += 1
                    d.pop(0)()

            col0 = 0
            n_t = len(TFS)
            LAG = 3   # mm2 js drip this many chunks behind relu
            for t, tf in enumerate(TFS):
                n_pp = (tf + CH - 1) // CH
                if t == 0:
                    # separate piece tiles: mm1 chunk k only waits its own
                    # piece's DMA (deps are tile-granular)
                    xqs = []
                    po = 0
                    for p in range(4):
                        sz = min(2048, tf - po)
                        xq = xin0.tile([128, 2048], f16, tag="xq",
                                       name="xq")
                        nc.sync.dma_start(xq[:, 0:sz],
                                          xt_d[:, col0 + po:col0 + po + sz])
                        xqs.append(xq)
                        po += sz
                else:
                    xt = xin.tile([128, 7680], f16, tag="xt", name="xt")
                    hw_ = (tf // (2 * CH)) * CH
                    nc.sync.dma_start(xt[:, 0:hw_],
                                      xt_d[:, col0:col0 + hw_])
                    nc.sync.dma_start(xt[:, hw_:tf],
                                      xt_d[:, col0 + hw_:col0 + tf])
                rlcs = []
                own = []
                pends.append(own)
                em = None
                for k in range(n_pp):
                    o = k * CH
                    w = min(CH, tf - o)
                    drip(2)
                    if t == 0:
                        xsrc, xo = xqs[k // 4], o - 2048 * (k // 4)
                    else:
                        xsrc, xo = xt, o
                    ppt = ppp.tile([128, 1024], f32, tag="pp", name="pp")
                    nc.tensor.matmul(ppt[:, 0:w], w1[0:64, :],
                                     xsrc[0:64, xo:xo + w],
                                     tile_position=(0, 0))
                    nc.tensor.matmul(ppt[:, 512:512 + w], w1[64:128, :],
                                     xsrc[64:128, xo:xo + w],
                                     tile_position=(64, 0))
                    src = ppt[:, 0:1024].rearrange(
                        "p (h k) -> p h k", h=2)[:, :, 0:w]
                    rlc = rlcp.tile([128, 1024], f16, tag="rlc",
                                    name="rlc")
                    rlcs.append(rlc)
                    dst = rlc[:, 0:1024].rearrange(
                        "p (h k) -> p h k", h=2)[:, :, 0:w]
                    if (pp_i % 31) % 2 == 0:
                        nc.scalar.activation(
                            dst, src, mybir.ActivationFunctionType.Relu)
                    else:
                        nc.vector.tensor_scalar_max(dst, src, 0.0)
                    pp_i += 1
                    # append own js at a LAG-chunk distance so dripped
                    # mm2s always read long-finished relu output
                    if em is None:
                        em, post = make_mm2(None, tf, t, rlcs=rlcs)
                    if k >= LAG:
                        own.extend(
                            [lambda j=j, em=em: em(j)
                             for j in (2 * (k - LAG), 2 * (k - LAG) + 1)
                             if j < J])
                own.extend([lambda j=j, em=em: em(j)
                            for j in range(max(0, 2 * (n_pp - LAG)), J)])
                own.extend([lambda h=h, post=post: post(h)
                            for h in (0, 1)])
                col0 += tf

            drip(10 ** 6)

    nc.compile()
    return nc


def kernel(x: np.ndarray, Wloc: np.ndarray, W: np.ndarray) -> np.ndarray:
    if "nc" not in _CACHE:
        _CACHE["nc"] = _build_kernel()
    nc = _CACHE["nc"]

    x = np.asarray(x, dtype=np.float32)
    # per-core halves; tile-scope j-major permute; transpose to [64, HALF]
    xh = x.reshape(N_CORES * 2, HALF, D_IN)
    parts = []
    o = 0
    for tf in TFS:
        gt = tf // J
        blk = xh[:, o:o + tf].reshape(-1, gt, J, D_IN).transpose(0, 2, 1, 3)
        parts.append(blk.reshape(N_CORES * 2, tf, D_IN))
        o += tf
    xp = np.concatenate(parts, axis=1)
    xp = xp.reshape(N_CORES, 2, HALF, D_IN).transpose(0, 1, 3, 2)
    xp = np.ascontiguousarray(xp, dtype=np.float16).reshape(N_CORES, 128, HALF)

    w1 = np.ascontiguousarray(
        np.concatenate([Wloc.T, Wloc.T], axis=0), dtype=np.float16)  # [128,128]
    w2 = np.ascontiguousarray((W / float(J)).T, dtype=np.float16)    # [128,10]

    in_maps = [{"xt": xp[c], "w1": w1, "w2": w2} for c in range(N_CORES)]
    res = run_bass_kernel_spmd(nc, in_maps, core_ids=list(range(N_CORES)))
    _CACHE["exec_time_ns"] = res.exec_time_ns
    _CACHE["trace"] = res.instructions_and_trace

    out = np.empty((L // J, C), dtype=np.float32)
    segs = L // J // N_CORES          # 5000
    segbase = np.cumsum([0] + [tf // J for tf in TFS])
    for c in range(N_CORES):
        oc = res.results[c]["out"]    # [10, 512*len(TFS)]
        base = c * segs
        for t, tf in enumerate(TFS):
            gt = tf // J
            blk = oc[:, 512 * t:512 * t + 2 * gt]
            for s in (0, 1):
                sb = base + s * SEG_HALF + segbase[t]
                out[sb:sb + gt] = blk[:, s * gt:(s + 1) * gt].T
    return out


# revision 49
# speedup vs baseline: 1.0663x; 1.0663x over previous
"""Trainium2 Bass kernel for segment-reduce classifier.

Reference computation:
    local = relu(x @ Wloc.T)            # [L, 128]
    feats = local.reshape(-1, 30, 128).mean(1)   # [L/30, 128]
    out   = feats @ W.T                 # [L/30, 10]

Strategy (8 NeuronCores, data-parallel on rows; 114.9us, 1.12x over the
129.1us 4-strip baseline; the ~88us/core PSUM-exit relu pass on the two
1-elem/lane/cycle engines is the architectural floor):
  - Per core xt [128, 75000] fp16: partitions 0-63 = rows[:75000].T ("A"),
    64-127 = rows[75000:].T ("B").  Within each outer tile (7680 rows =
    256 segments per stream; last 5880 = 196) the host pre-permutes rows
    to tile-scope j-major (r = j*gt + g), so every on-chip access pattern
    is contiguous (strided matmul operands measured ~4x slow; fp8e3
    moving operands 2 cyc/col - hence fp16 and host-side reordering).
  - mm1: each [128, 1024] PSUM tile holds one A-chunk (bank 0, PE rows
    0-63) and one B-chunk (bank 1, rows 64-127); the two matmuls are
    adjacent in the queue and run concurrently on disjoint row quadrants.
  - relu PSUM fp32 -> SBUF fp16 into per-chunk [128, 1024] rl tiles, one
    flat instruction per chunk, engines alternating Scalar(16) :
    Vector(15) - both ~1 elem/lane/cycle on fp32 PSUM reads; this pass
    is the roofline, so everything else hides under it.
  - mm2 (mean-pool + classifier fused): per j, FOUR concurrent 128-col
    matmuls on PE column groups 32q accumulate into one PSUM bank at
    disjoint partition ranges (per-element has_written semantics make
    interleaved accumulation groups safe).  Pooling is free PSUM
    accumulation; 1/30 folded into W.  A j-slice lives entirely in rl
    chunk (j*gt)//512 (straddle-split for the 196-segment last tile), so
    mm2 js drip into the emission stream three chunks behind relu - the
    in-order PE queue interleaves them between mm1 pairs without ever
    waiting, and no backlog spills past the last relu.
  - Per tile: one [128, 2*gt] copy PSUM->SBUF (alternating engines) + 4
    column-group DMAs straight to DRAM; host reassembles.
"""

import numpy as np

import concourse.bacc as bacc
import concourse.bass as bass
import concourse.tile as tile
from concourse import mybir
from concourse.bass_utils import run_bass_kernel_spmd

# Problem constants (hardcoded per harness contract)
L, D_IN, D_ENC, C, J = 1200000, 64, 128, 10, 30
N_CORES = 8
R = L // N_CORES          # rows per core = 150000
HALF = R // 2             # 75000 rows per quadrant stream
SEG_HALF = HALF // J      # 2500 segments per stream
TFS = [7680] * 9 + [5880]  # rows per outer tile per stream
CH = 512                  # rows per PSUM bank chunk

_CACHE = {}


def _build_kernel():
    nc = bacc.Bacc("TRN2", target_bir_lowering=False, debug=False,
                   num_devices=N_CORES)
    f32, f16 = mybir.dt.float32, mybir.dt.float16

    xt_d = nc.dram_tensor("xt", [128, HALF], f16, kind="ExternalInput")
    w1_d = nc.dram_tensor("w1", [128, D_ENC], f16, kind="ExternalInput")
    w2_d = nc.dram_tensor("w2", [128, C], f16, kind="ExternalInput")
    # tile t at cols [512t, 512t + 2*gt); half h at h*gt + seg
    out_d = nc.dram_tensor("out", [C, 512 * len(TFS)], f32,
                           kind="ExternalOutput")

    with tile.TileContext(nc) as tc:
        with (
            tc.tile_pool(name="consts", bufs=1) as consts,
            tc.tile_pool(name="xin", bufs=4) as xin,
            tc.tile_pool(name="rlcp", bufs=10) as rlcp,
            tc.tile_pool(name="stgp", bufs=2) as stgp,
            tc.tile_pool(name="ppp", bufs=3, space="PSUM") as ppp,
            tc.tile_pool(name="accp", bufs=2, space="PSUM") as accp,
        ):
            w1 = consts.tile([128, D_ENC], f16)
            nc.sync.dma_start(w1[:], w1_d[:])
            w2 = consts.tile([128, C], f16)
            nc.sync.dma_start(w2[:], w2_d[:])

            pp_i = 0       # global relu counter (engine pattern, 8A:7D)
            pends = []     # lists of deferred mm2 emitters, one per tile
            rr = [0]       # round-robin cursor over pends

            def make_mm2(rl_t, tf, t, rlcs=None):
                gt = tf // J
                hgt = gt // 2
                if rlcs is None:
                    rlh = rl_t.rearrange("p (h q) -> p h q", h=2)
                box = {}

                def emit(j):
                    if j == 0:
                        box["acc"] = accp.tile([128, 512], f32, tag="acc",
                                               name="acc")
                    acc = box["acc"]
                    # 4 concurrent column-group quarters per j; out col
                    # q*hgt == h*gt + (q%2)*hgt, same layout as one wide mm
                    for q in range(4):
                        so = j * gt + (q % 2) * hgt
                        if rlcs is None:
                            nc.tensor.matmul(
                                acc[32 * q:32 * q + C,
                                    q * hgt:(q + 1) * hgt],
                                w2[:], rlh[:, q // 2, so:so + hgt],
                                start=(j == 0), stop=(j == J - 1),
                                tile_position=(0, 32 * q),
                                skip_group_check=True)
                        else:
                            # per-chunk rl tiles; reads may straddle banks
                            a, outo, left = so, q * hgt, hgt
                            while left > 0:
                                ck, off = a // CH, a % CH
                                take = min(left, CH - off)
                                nc.tensor.matmul(
                                    acc[32 * q:32 * q + C,
                                        outo:outo + take],
                                    w2[:],
                                    rlcs[ck][:, (q // 2) * CH + off:
                                             (q // 2) * CH + off + take],
                                    start=(j == 0), stop=(j == J - 1),
                                    tile_position=(0, 32 * q),
                                    skip_group_check=True)
                                a += take
                                outo += take
                                left -= take
                    if j == J - 1:
                        stg = stgp.tile([128, 512], f32, tag="stg",
                                        name="stg")
                        if t % 2 == 1:
                            nc.scalar.copy(stg[:, 0:2 * gt],
                                           acc[:, 0:2 * gt])
                        else:
                            nc.vector.tensor_scalar_add(
                                stg[:, 0:2 * gt], acc[:, 0:2 * gt], 0.0)
                        for q in range(4):
                            nc.sync.dma_start(
                                out_d[:, 512 * t + q * hgt:
                                      512 * t + (q + 1) * hgt],
                                stg[32 * q:32 * q + C,
                                    q * hgt:(q + 1) * hgt])
                    return
                return emit

            def drip(n):
                for _ in range(n):
                    live = [d for d in pends if d]
                    if not live:
                        return
                    d = live[rr[0] % len(live)]
                    rr[0] += 1
                    d.pop(0)()

            col0 = 0
            n_t = len(TFS)
            LAG = 3   # mm2 js drip this many chunks behind relu
            for t, tf in enumerate(TFS):
                xt = xin.tile([128, 7680], f16, tag="xt", name="xt")
                # split DMA: lets mm1 start on the first piece sooner
                nsp = 4 if t == 0 else 2
                bounds = [(tf // (nsp * CH)) * CH * i for i in range(nsp)]
                bounds.append(tf)
                for b0, b1 in zip(bounds, bounds[1:]):
                    if b1 > b0:
                        nc.sync.dma_start(xt[:, b0:b1],
                                          xt_d[:, col0 + b0:col0 + b1])

                n_pp = (tf + CH - 1) // CH
                rlcs = []
                own = []
                pends.append(own)
                em = None
                for k in range(n_pp):
                    o = k * CH
                    w = min(CH, tf - o)
                    drip(2)
                    ppt = ppp.tile([128, 1024], f32, tag="pp", name="pp")
                    nc.tensor.matmul(ppt[:, 0:w], w1[0:64, :],
                                     xt[0:64, o:o + w],
                                     tile_position=(0, 0))
                    nc.tensor.matmul(ppt[:, 512:512 + w], w1[64:128, :],
                                     xt[64:128, o:o + w],
                                     tile_position=(64, 0))
                    src = ppt[:, 0:1024].rearrange(
                        "p (h k) -> p h k", h=2)[:, :, 0:w]
                    rlc = rlcp.tile([128, 1024], f16, tag="rlc",
                                    name="rlc")
                    rlcs.append(rlc)
                    dst = rlc[:, 0:1024].rearrange(
                        "p (h k) -> p h k", h=2)[:, :, 0:w]
                    if (pp_i % 31) % 2 == 0:
                        nc.scalar.activation(
                            dst, src, mybir.ActivationFunctionType.Relu)
                    else:
                        nc.vector.tensor_scalar_max(dst, src, 0.0)
                    pp_i += 1
                    # append own js at a LAG-chunk distance so dripped
                    # mm2s always read long-finished relu output
                    if em is None:
                        em = make_mm2(None, tf, t, rlcs=rlcs)
                    if k >= LAG:
                        own.extend(
                            [lambda j=j, em=em: em(j)
                             for j in (2 * (k - LAG), 2 * (k - LAG) + 1)
                             if j < J])
                own.extend([lambda j=j, em=em: em(j)
                            for j in range(max(0, 2 * (n_pp - LAG)), J)])
                col0 += tf

            drip(10 ** 6)

    nc.compile()
    return nc


def kernel(x: np.ndarray, Wloc: np.ndarray, W: np.ndarray) -> np.ndarray:
    if "nc" not in _CACHE:
        _CACHE["nc"] = _build_kernel()
    nc = _CACHE["nc"]

    x = np.asarray(x, dtype=np.float32)
    # per-core halves; tile-scope j-major permute; transpose to [64, HALF]
    xh = x.reshape(N_CORES * 2, HALF, D_IN)
    parts = []
    o = 0
    for tf in TFS:
        gt = tf // J
        blk = xh[:, o:o + tf].reshape(-1, gt, J, D_IN).transpose(0, 2, 1, 3)
        parts.append(blk.reshape(N_CORES * 2, tf, D_IN))
        o += tf
    xp = np.concatenate(parts, axis=1)
    xp = xp.reshape(N_CORES, 2, HALF, D_IN).transpose(0, 1, 3, 2)
    xp = np.ascontiguousarray(xp, dtype=np.float16).reshape(N_CORES, 128, HALF)

    w1 = np.ascontiguousarray(
        np.concatenate([Wloc.T, Wloc.T], axis=0), dtype=np.float16)  # [128,128]
    w2 = np.ascontiguousarray((W / float(J)).T, dtype=np.float16)    # [128,10]

    in_maps = [{"xt": xp[c], "w1": w1, "w2": w2} for c in range(N_CORES)]
    res = run_bass_kernel_spmd(nc, in_maps, core_ids=list(range(N_CORES)))
    _CACHE["exec_time_ns"] = res.exec_time_ns
    _CACHE["trace"] = res.instructions_and_trace

    out = np.empty((L // J, C), dtype=np.float32)
    segs = L // J // N_CORES          # 5000
    segbase = np.cumsum([0] + [tf // J for tf in TFS])
    for c in range(N_CORES):
        oc = res.results[c]["out"]    # [10, 512*len(TFS)]
        base = c * segs
        for t, tf in enumerate(TFS):
            gt = tf // J
            blk = oc[:, 512 * t:512 * t + 2 * gt]
            for s in (0, 1):
                sb = base + s * SEG_HALF + segbase[t]
                out[sb:sb + gt] = blk[:, s * gt:(s + 1) * gt].T
    return out
